# revision 33
# baseline (speedup 1.0000x reference)
"""Trainium2 Bass kernel for nn_DAGConcat (DAG-GNN + LSTM + MLP).

Sharding: data-parallel over B=32 dialogues across 8 cores (4 each).

v2 structure: the two GNN layer recurrences run INTERLEAVED (layer 1 lags
layer 0 by KLAG steps, fed by chunked pre-projections), and the LSTM's 32
steps are stitched into the same instruction stream as wide-matmul filler
that keeps the PE HAM clock-gate at 8/8.  Gate pointwise math is packed
into 36-row ops spread across DVE/ACT/Pool.
"""
import os
import sys

for _p in ('/opt/trn_rl_repo', '/root/.axon_site/_ro/trn_rl_repo'):
    if os.path.isdir(_p) and _p not in sys.path:
        sys.path.insert(0, _p)

import ml_dtypes
import numpy as np
import concourse.bass as bass
import concourse.mybir as mybir
import concourse.tile as tile
from concourse.bass_utils import run_bass_kernel_spmd
from concourse.masks import make_identity

F32 = mybir.dt.float32
BF16 = mybir.dt.bfloat16
FP8 = mybir.dt.float8e4
NPF8 = ml_dtypes.float8_e4m3
I32 = mybir.dt.int32
AF = mybir.ActivationFunctionType
ALU = mybir.AluOpType
AX = mybir.AxisListType

B, N, EMB, HID, L, NCLS = 32, 128, 1024, 512, 2, 7
NCORES = 8
BL = B // NCORES
H4, H2 = 4 * HID, 2 * HID
PRE = 6 * HID           # pre-MM cols: rC|rP|zC|zP|nCq|nPq
DEBUG = bool(int(os.environ.get('KDEBUG', '0')))
KSKIP_LSTM = bool(int(os.environ.get('KSKIP_LSTM', '0')))
KSKIP_MLP = bool(int(os.environ.get('KSKIP_MLP', '0')))
KNSTEP = int(os.environ.get('KNSTEP', str(N)))
KLAG = int(os.environ.get('KLAG', '34'))
KWARM = int(os.environ.get('KWARM', '2'))
KLSTMY = int(os.environ.get('KLSTMY', '3'))
CH = 32                 # pre-chunk size for layer 1


def _patch_drain():
    """Kernel-tail drain waits on >limit sems -> emit 1-wait drains instead."""
    from concourse.vector_clock import ScopedClock, VectorClock

    def _drain_and_barrier(self, tick_clock, wait_clock):
        gc = tick_clock.global_clock
        n = len(gc)
        for st in range(n):
            if gc[st] <= 0:
                continue
            vec = [0] * n
            vec[st] = gc[st]
            di = self.nc.sync.drain()
            wait_clock.add_sem_waits(di.ins, ScopedClock({None: VectorClock(vec)}))
        self.nc.all_engine_barrier()
        popped = self.nc._tile_sem_poison_stack.pop()
        assert popped is self._sem_poison
        self.nc.clear_and_free_semaphores(list(self.sems.allocated().values()))
        self.nc.all_engine_barrier()

    tile.TileContext._drain_and_barrier = _drain_and_barrier


_patch_drain()


# ================================================================ host prep

def prep_inputs(inp):
    f4 = np.float32
    feats = np.asarray(inp['features'], f4)
    adj = np.asarray(inp['adj'], f4)
    s_mask = np.asarray(inp['s_mask'], f4)

    g = {}
    # LSTM: gate col order [i f o g] so sigma gates are contiguous
    # lstm gate order f, i, g, o (torch rows are i, f, g, o)
    perm = np.concatenate([np.arange(HID, 2 * HID), np.arange(0, HID),
                           np.arange(2 * HID, 3 * HID), np.arange(3 * HID, 4 * HID)])
    g['lstm_WihT'] = np.ascontiguousarray(np.asarray(inp['lstm_Wih'], f4).T[:, perm])
    g['lstm_WhhT'] = np.ascontiguousarray(np.asarray(inp['lstm_Whh'], f4).T[:, perm])
    g['lstm_b'] = (np.asarray(inp['lstm_bih'], f4)
                   + np.asarray(inp['lstm_bhh'], f4))[perm][None, :]
    g['fc1T'] = np.ascontiguousarray(np.asarray(inp['fc1_W'], f4).T)
    g['fc1_b'] = np.asarray(inp['fc1_b'], f4)[None, :]

    abrow = np.zeros((1, 2), f4)
    for l in range(L):
        aW = np.asarray(inp['attn_W'][l], f4)
        wq, wk = aW[:HID], aW[HID:]
        abrow[0, l] = float(np.asarray(inp['attn_b'], f4)[l])
        cWihT = np.asarray(inp['gruC_Wih'][l], f4).T
        cWhhT = np.asarray(inp['gruC_Whh'][l], f4).T
        cbih = np.asarray(inp['gruC_bih'][l], f4)
        cbhh = np.asarray(inp['gruC_bhh'][l], f4)
        pWihT = np.asarray(inp['gruP_Wih'][l], f4).T
        pWhhT = np.asarray(inp['gruP_Whh'][l], f4).T
        pbih = np.asarray(inp['gruP_bih'][l], f4)
        pbhh = np.asarray(inp['gruP_bhh'][l], f4)
        Wr0 = np.asarray(inp['Wr0'][l], f4)
        Wr1 = np.asarray(inp['Wr1'][l], f4)
        r, z, n_ = slice(0, HID), slice(HID, 2 * HID), slice(2 * HID, 3 * HID)
        # gate-sigma weight, band order zC | zP | rC | rP (h-side), fp8
        g[f'Wc1_{l}'] = np.ascontiguousarray(np.concatenate(
            [cWhhT[:, z], pWihT[:, z], cWhhT[:, r], pWihT[:, r]],
            axis=1)).astype(NPF8)
        # n-gate weight (h-side): nC | nP, fp8
        g[f'Wc2_{l}'] = np.ascontiguousarray(np.concatenate(
            [cWhhT[:, n_], pWihT[:, n_]], axis=1)).astype(NPF8)
        g[f'bias2_{l}'] = np.concatenate([cbhh[n_], pbih[n_]])[None, :]
        # V-projection + (-wk) + wq
        g[f'Wr_{l}'] = np.ascontiguousarray(np.concatenate(
            [Wr0.T, Wr1.T, -wk[:, None], wq[:, None]], axis=1))
        # q-side pre blocks: zC | zP | rC | rP | nCq | nPq
        g[f'Wpre_{l}'] = np.ascontiguousarray(np.concatenate(
            [cWihT[:, z], pWhhT[:, z], cWihT[:, r], pWhhT[:, r],
             cWihT[:, n_], pWhhT[:, n_]], axis=1))
        g[f'biaspre_{l}'] = np.concatenate(
            [cbih[z] + cbhh[z], pbih[z] + pbhh[z], cbih[r] + cbhh[r],
             pbih[r] + pbhh[r], cbih[n_], pbhh[n_]])[None, :].astype(f4)
    g['abrow'] = abrow

    g['mlp0T'] = np.ascontiguousarray(np.asarray(inp['mlp0_W'], f4).T)
    g['mlp0_b'] = np.asarray(inp['mlp0_b'], f4)[None, :]
    g['mlp1T'] = np.ascontiguousarray(np.asarray(inp['mlp1_W'], f4).T)
    g['mlp1_b'] = np.asarray(inp['mlp1_b'], f4)[None, :]
    ow = np.zeros((HID, 8), f4)
    ow[:, :NCLS] = np.asarray(inp['out_W'], f4).T
    g['outWT'] = ow
    ob = np.zeros((1, 8), f4)
    ob[0, :NCLS] = np.asarray(inp['out_b'], f4)
    g['out_b'] = ob

    featT = np.ascontiguousarray(feats.transpose(2, 0, 1))  # [EMB, B, N]
    g['featT_full'] = featT

    maps = []
    for c in range(NCORES):
        bs = slice(BL * c, BL * (c + 1))
        m = dict(g)
        m['featT_l'] = np.ascontiguousarray(featT[:, bs, :])
        m['absr'] = np.ascontiguousarray(np.concatenate(
            [(adj[bs] - 1.0) * 1e30, s_mask[bs]], axis=2))
        m['town'] = np.arange(BL * c, BL * (c + 1), dtype=np.int32)[None, :]
        maps.append(m)
    return maps


SHAPES = {
    'lstm_WihT': (EMB, 4 * HID), 'lstm_WhhT': (HID, 4 * HID), 'lstm_b': (1, 4 * HID),
    'fc1T': (EMB, HID), 'fc1_b': (1, HID), 'abrow': (1, 2),
    'mlp0T': (4 * HID + EMB, HID), 'mlp0_b': (1, HID),
    'mlp1T': (HID, HID), 'mlp1_b': (1, HID), 'outWT': (HID, 8), 'out_b': (1, 8),
    'featT_full': (EMB, B, N), 'featT_l': (EMB, BL, N),
    'absr': (BL, N, 2 * N), 'town': (1, BL),
}
for _l in range(L):
    SHAPES[f'Wc1_{_l}'] = (HID, H4)
    SHAPES[f'Wc2_{_l}'] = (HID, H2)
    SHAPES[f'bias2_{_l}'] = (1, H2)
    SHAPES[f'Wr_{_l}'] = (HID, H2 + 2)
    SHAPES[f'Wpre_{_l}'] = (HID, PRE)
    SHAPES[f'biaspre_{_l}'] = (1, PRE)


# ================================================================ device build

def _loadw(nc, pool, dram, kdim, fdim, tag, dtype=BF16):
    kc = kdim // 128
    t = pool.tile([128, kc, fdim], dtype, tag=tag)
    nd = len(dram.shape)
    if nd == 2:
        src = dram[:].rearrange("(c p) f -> p c f", p=128)
    elif nd == 3:
        src = dram[:].rearrange("(c p) a b -> p c (a b)", p=128)
    else:
        raise ValueError(nd)
    nc.gpsimd.dma_start(t[:], src)
    return t


def _diag(Wz):
    """Diagonal columns {0,5,10,15} of a [128,16] tile as a [128,4] AP."""
    ap = Wz[:]
    return bass.AP(tensor=ap.tensor, offset=ap.offset,
                   ap=[ap.ap[0], [5, 4]])


WAIT_CAP = {}


def _cap_waits(nc):
    """Split excess semaphore waits onto same-engine NOPs (HW wait-slot caps)."""
    for f in nc.m.functions:
        for bb in f.blocks:
            newlist = []
            for ins in bb.instructions:
                si = getattr(ins, 'sync_info', None)
                waits = list(si.on_wait) if si and si.on_wait else []
                cap = WAIT_CAP.get(type(ins).__name__, 1)
                if len(waits) > cap:
                    excess = waits[:-cap] if cap > 0 else waits
                    keep = waits[-cap:] if cap > 0 else []
                    for w in excess:
                        nop = mybir.InstNoOp(
                            name=nc.get_next_instruction_name(),
                            text_hint='wait_spill', bass_nofuse=True)
                        nop.engine = ins.engine
                        nop.sync_info = mybir.SyncInfo(on_wait=[w], on_update=[])
                        nc.register_instruction(nop, overwrite=True)
                        newlist.append(nop)
                    si.on_wait = keep
                    ins.sync_info = si
                newlist.append(ins)
            bb.instructions = newlist


def build_nc():
    nc = bass.Bass()
    din = {}
    for name, shp in SHAPES.items():
        dt = I32 if name == 'town' else (
            FP8 if name.startswith(('Wc1_', 'Wc2_')) else F32)
        din[name] = nc.dram_tensor(name, list(shp), dt, kind="ExternalInput")
    out_dram = nc.dram_tensor('out', [BL, N, NCLS], F32, kind="ExternalOutput")
    dbg_dram = (nc.dram_tensor('dbg', [L, N, BL, HID], F32, kind="ExternalOutput")
                if DEBUG else None)

    pre_dram = [nc.dram_tensor(f'pre_dram{l}', [N, 24, HID], BF16) for l in range(L)]
    qpre_dram = [nc.dram_tensor(f'qpre_dram{l}', [N, BL], F32) for l in range(L)]
    hq_dram = [nc.dram_tensor(f'hq_dram{l}', [N, BL, HID], BF16) for l in range(L)]
    lstmT_dram = nc.dram_tensor('lstmT_dram', [B, 128, 4, N], BF16)

    with tile.TileContext(nc) as tc:  # noqa: SIM117
        with tc.tile_pool(name="state", bufs=1) as state, \
             tc.tile_pool(name="wperm", bufs=1) as wperm:

            ident = state.tile([128, 128], BF16, tag='ident')
            make_identity(nc, ident[:])
            ones_row = state.tile([1, 128], BF16, tag='ones')
            nc.vector.memset(ones_row[:], 1.0)
            ones4c = state.tile([128, 1], BF16, tag='ones4c')
            nc.vector.memset(ones4c[:], 1.0)
            zrow = state.tile([128, HID], BF16, tag='zrow')
            nc.vector.memset(zrow[:], 0.0)

            # ---------- persistent weights (both layers) ----------
            Wc1 = [_loadw(nc, wperm, din[f'Wc1_{l}'], HID, H4, f'Wc1_{l}',
                          dtype=FP8) for l in range(L)]
            Wc2 = [_loadw(nc, wperm, din[f'Wc2_{l}'], HID, H2, f'Wc2_{l}',
                          dtype=FP8) for l in range(L)]
            Wrt = [_loadw(nc, wperm, din[f'Wr_{l}'], HID, H2 + 2, f'Wr_{l}')
                   for l in range(L)]
            b2 = []
            for l in range(L):
                t = wperm.tile([1, H2], BF16, tag=f'b2_{l}', name=f'b2_{l}')
                nc.gpsimd.dma_start(t[:], din[f'bias2_{l}'][:])
                b2.append(t)
            abt_row = wperm.tile([1, 2], BF16, tag='abrow')
            nc.gpsimd.dma_start(abt_row[:], din['abrow'][:])

            # ---------- persistent state ----------
            HT = [state.tile([128, 4, BL, N], BF16, tag=f'HT{k}', name=f'HT{k}')
                  for k in range(3)]
            for k in range(3):
                nc.vector.memset(HT[k][:], 0.0)
            V01 = [state.tile([128, BL, H2], BF16, tag=f'V01_{l}',
                              name=f'V01_{l}') for l in range(L)]
            Kneg = [state.tile([BL, N], F32, tag=f'Kneg_{l}', name=f'Kneg_{l}')
                    for l in range(L)]
            ew0 = [state.tile([BL, N], BF16, tag=f'ew0_{l}', name=f'ew0_{l}')
                   for l in range(L)]
            ew1 = [state.tile([BL, N], BF16, tag=f'ew1_{l}', name=f'ew1_{l}')
                   for l in range(L)]
            Wz0d = [state.tile([128, 16], BF16, tag=f'Wz0_{l}', name=f'Wz0_{l}')
                    for l in range(L)]
            Wz1d = [state.tile([128, 16], BF16, tag=f'Wz1_{l}', name=f'Wz1_{l}')
                    for l in range(L)]
            for l in range(L):
                nc.vector.memset(V01[l][:], 0.0)
                nc.vector.memset(Kneg[l][:], 0.0)
                nc.vector.memset(ew0[l][:], 0.0)
                nc.vector.memset(ew1[l][:], 0.0)
                nc.vector.memset(Wz0d[l][:], 0.0)
                nc.vector.memset(Wz1d[l][:], 0.0)

            # ---------- prologue: H0 + pre(0) ----------
            with tc.tile_pool(name="prol", bufs=1) as prol, \
                 tc.tile_pool(name="ps1", bufs=2, space="PSUM") as ps1, \
                 tc.tile_pool(name="pstep", bufs=2) as pstep:
                featT = _loadw(nc, prol, din['featT_l'], EMB, BL * N, 'featT')
                featT4 = featT[:].rearrange("p c (b n) -> p c b n", b=BL)
                fc1T = _loadw(nc, prol, din['fc1T'], EMB, HID, 'fc1T')
                fc1b = prol.tile([1, HID], BF16, tag='fc1b')
                nc.gpsimd.dma_start(fc1b[:], din['fc1_b'][:])

                _h0_phase(nc, tc, ps1, pstep, featT4, fc1T, fc1b, ones_row,
                          ident, HT[0], hq_dram[0])
                for c in range(4):
                    _pre_chunk(nc, ps1, pstep, HT[0], din['Wpre_0'],
                               din['biaspre_0'], Wrt[0], abt_row, 0,
                               ones_row, pre_dram[0], qpre_dram[0], c)

            # ---------- merged loop ----------
            with tc.tile_pool(name="wlstm", bufs=1) as wlstm, \
                 tc.tile_pool(name="step", bufs=2) as step, \
                 tc.tile_pool(name="dma2", bufs=3) as dma2, \
                 tc.tile_pool(name="gA", bufs=1, space="PSUM") as gA, \
                 tc.tile_pool(name="gB", bufs=1, space="PSUM") as gB, \
                 tc.tile_pool(name="gV", bufs=1, space="PSUM") as gV, \
                 tc.tile_pool(name="fill", bufs=1, space="PSUM") as fill, \
                 tc.tile_pool(name="aux", bufs=2, space="PSUM") as aux:

                axb = aux.tile([128, 1024], BF16, tag='axb', bufs=1)
                wrm = aux.tile([128, 512], F32, tag='wrm', bufs=1)
                if KSKIP_LSTM:
                    lstm_gen = iter(())
                else:
                    lstm_gen = _lstm_steps(nc, din, wlstm, state, step, dma2,
                                           fill, axb, ones_row, ident,
                                           lstmT_dram)

                ctx = dict(nc=nc, din=din, step=step, dma2=dma2, gA=gA, gB=gB,
                           gV=gV, axb=axb, ones4c=ones4c, zrow=zrow,
                           qpre_dram=qpre_dram, ident=ident,
                           ones_row=ones_row,
                           abt_row=abt_row, HT=HT, V01=V01, Kneg=Kneg,
                           ew0=ew0, ew1=ew1, Wz0d=Wz0d, Wz1d=Wz1d,
                           Wc1=Wc1, Wc2=Wc2, Wrt=Wrt, b2=b2,
                           pre_dram=pre_dram, hq_dram=hq_dram,
                           dbg_dram=dbg_dram)

                def drive(*gens):
                    live, out = [g for g in gens if g is not None], [None] * 9
                    while live:
                        for g in list(live):
                            r = next(g, '_end_')
                            if r == '_end_':
                                live.remove(g)
                            elif r is not None:
                                out[gens.index(g)] = r
                    return out

                nsteps = min(N, KNSTEP)
                total_s = nsteps + KLAG
                ly = 0  # lstm yields consumed
                pend1 = None  # layer-1 H1 tile-dict awaiting its H2
                for s in range(total_s):
                    j = s - KLAG
                    g_h1_0 = _rec_h1(ctx, 0, s) if s < nsteps else None
                    g_h2_1 = (_rec_h2(ctx, 1, j, pend1)
                              if 0 <= j < nsteps else None)
                    h1_0 = drive(g_h1_0, g_h2_1)[0]
                    # lstm pacing
                    target = (s + 1) * 340 // max(total_s - 8, 1) * KLSTMY // 3
                    while ly < target:
                        if next(lstm_gen, 'done') == 'done':
                            ly = 10**9
                            break
                        ly += 1
                    for _ in range(KWARM):
                        nc.tensor.matmul(wrm[:], ident[:], Wrt[0][:, 0, 0:512],
                                         start=True, stop=True,
                                         skip_group_check=True)
                    j1 = s - KLAG + 1
                    g_h1_1 = _rec_h1(ctx, 1, j1) if 0 <= j1 < nsteps else None
                    g_h2_0 = (_rec_h2(ctx, 0, s, h1_0)
                              if s < nsteps else None)
                    pend1_new = drive(g_h1_1, g_h2_0)[0]
                    if g_h1_1 is not None:
                        pend1 = pend1_new
                    # pre(1) chunk once its 32 rows of H1 are all written
                    if s < nsteps and s % CH == CH - 1:
                        _pre_chunk(nc, fill, step, HT[1], din['Wpre_1'],
                                   din['biaspre_1'], Wrt[1], abt_row, 1,
                                   ones_row, pre_dram[1], qpre_dram[1],
                                   s // CH)
                # drain remaining lstm
                for _ in lstm_gen:
                    pass

            # ---------- final MLP ----------
            with tc.tile_pool(name="wmlp", bufs=1) as wmlp, \
                 tc.tile_pool(name="mstep", bufs=2) as mstep, \
                 tc.tile_pool(name="psm", bufs=2, space="PSUM") as psm:
                # own-rows gather: lstmT_dram[town[b]] -> lstmTl[:, :, b, :]
                lstmTl = wmlp.tile([128, 4, BL, N], BF16, tag='lstmTl')
                town_sb = wmlp.tile([1, BL], I32, tag='town')
                nc.gpsimd.dma_start(town_sb[:], din['town'][:])
                if KSKIP_LSTM:
                    nc.vector.memset(lstmTl[:], 0.0)
                else:
                    _, tvals = nc.values_load_multi_w_load_instructions(
                        town_sb[0:1, :], engines=[mybir.EngineType.Pool],
                        min_val=0, max_val=B - 1, skip_runtime_bounds_check=True)
                    for b in range(BL):
                        lsrc = lstmT_dram[bass.ds(tvals[b], 1), :, :, :]
                        nc.gpsimd.dma_start(lstmTl[:, :, b, :], lsrc)
                if KSKIP_MLP:
                    for b in range(BL):
                        nc.gpsimd.dma_start(out_dram[b, :, :],
                                            ident[0:128, 0:NCLS])
                else:
                    featTm = _loadw(nc, wmlp, din['featT_l'], EMB, BL * N,
                                    'featTm')
                    featT4m = featTm[:].rearrange("p c (b n) -> p c b n", b=BL)
                    _final_mlp(nc, tc, din, wmlp, mstep, psm, featT4m, HT,
                               lstmTl, ones_row, ident, out_dram)
    _cap_waits(nc)
    return nc


def _h0_phase(nc, tc, ps, step, featT4, fc1T, fc1b, ones_row, ident, HT0,
              hq0_dram):
    for b in range(BL):
        p = ps.tile([128, HID], F32, tag='h0ps')
        for k in range(8):
            nc.tensor.matmul(p[:], featT4[:, k, b, :], fc1T[:, k, :],
                             start=(k == 0), stop=False)
        nc.tensor.matmul(p[:], ones_row[0:1, 0:128], fc1b[:],
                         start=False, stop=True)
        h0 = step.tile([128, HID], BF16, tag='h0sb', bufs=2)
        nc.scalar.activation(h0[:], p[:], AF.Relu)
        for c in range(4):
            tp = ps.tile([128, 128], BF16, tag='h0tp')
            nc.tensor.transpose(tp[:], h0[:, c * 128:(c + 1) * 128], ident[:])
            nc.vector.tensor_copy(HT0[:, c, b, :], tp[:])
        nc.sync.dma_start(hq0_dram[:, b, :], h0[:])


def _pre_chunk(nc, ps, step, HTl, Wpre_dram, bpre_dram, Wrt, abt_row, l,
               ones_row, pre_dram_l, qpre_dram_l, c):
    """Pre-projections for rows [32c, 32c+32) of HTl, all 4 dialogues packed
    into the 128 output partitions (b-major, n-minor).  Wpre is streamed from
    DRAM per 512-col block to keep SBUF small."""
    c0 = c * CH
    # stage the (b, n)-packed transposed H rows contiguously
    hstg = step.tile([128, 4, 128], BF16, tag='hstg', bufs=2)
    for k in range(4):
        nc.vector.tensor_copy(hstg[:, k, :], HTl[:, k, :, c0:c0 + CH])
    eng = [nc.scalar, nc.vector, nc.vector]
    for blk in range(6):
        sl = slice(blk * HID, (blk + 1) * HID)
        bp = step.tile([1, HID], BF16, tag='bp', bufs=2)
        nc.gpsimd.dma_start(bp[:], bpre_dram[0:1, sl])
        wpb = step.tile([128, 4, HID], BF16, tag='wpb', bufs=2)
        nc.gpsimd.dma_start(
            wpb[:], Wpre_dram[:, sl].rearrange("(c p) f -> p c f", p=128))
        p = ps.tile([128, HID], F32, tag='fg')
        nc.tensor.matmul(p[:], ones_row[0:1, :], bp[:],
                         start=True, stop=False)
        for k in range(4):
            nc.tensor.matmul(p[:], hstg[:, k, :], wpb[:, k, :],
                             start=False, stop=(k == 3))
        sb = step.tile([128, HID], BF16, tag='psb', bufs=1)
        if blk % 3 == 0:
            nc.scalar.activation(sb[:], p[:], AF.Copy)
        else:
            eng[blk % 3].tensor_copy(sb[:], p[:])
        for b in range(BL):
            nc.sync.dma_start(pre_dram_l[c0:c0 + CH, 4 * blk + b, :]
                              if blk < 4 else
                              pre_dram_l[c0:c0 + CH, 16 + 4 * (blk - 4) + b, :],
                              sb[CH * b:CH * (b + 1), :])
    # Qpre column: q @ wq + ab  ->  qpre_dram[c0:c0+CH, b]
    pq = ps.tile([128, HID], F32, tag='fg')
    nc.tensor.matmul(pq[:, 0:1], ones_row[0:1, :], abt_row[:, l:l + 1],
                     start=True, stop=False)
    for k in range(4):
        nc.tensor.matmul(pq[:, 0:1], hstg[:, k, :], Wrt[:, k, H2 + 1:H2 + 2],
                         start=False, stop=(k == 3))
    sbq = step.tile([128, 1], F32, tag='sbq', bufs=2)
    nc.vector.tensor_copy(sbq[:], pq[:, 0:1])
    for b in range(BL):
        nc.sync.dma_start(qpre_dram_l[c0:c0 + CH, b:b + 1],
                          sbq[CH * b:CH * (b + 1), :])


def _lstm_steps(nc, din, wpool, state, step, dma2, fill, axb, ones_row, ident,
                lstmT_dram):
    """Generator emitting the 32 sequential LSTM steps in ~10 pieces each."""
    WihT = _loadw(nc, wpool, din['lstm_WihT'], EMB, 4 * HID, 'lWih')
    WhhT = _loadw(nc, wpool, din['lstm_WhhT'], HID, 4 * HID, 'lWhh')
    lb = wpool.tile([1, 4 * HID], BF16, tag='lb')
    nc.gpsimd.dma_start(lb[:], din['lstm_b'][:])

    hT = state.tile([128, 4, 128], BF16, tag='lhT')     # h_{t-1} transposed
    cst = state.tile([128, HID], F32, tag='lc')
    nc.vector.memset(hT[:], 0.0)
    nc.vector.memset(cst[:], 0.0)

    ft = dma2.tile([128, 8, 128], BF16, tag='lft', bufs=1)
    nc.gpsimd.dma_start(
        ft[:], din['featT_full'][:, 0, :].rearrange("(c p) n -> p c n", p=128))
    for t in range(B):
        # gate block order: f, i, g, o (two rotating sig tiles)
        tiles = {}
        for blk in range(4):
            sl = slice(blk * HID, (blk + 1) * HID)
            g = fill.tile([128, HID], F32, tag='fg')
            nc.tensor.matmul(g[:], ones_row[0:1, 0:128], lb[:, sl],
                             start=True, stop=False)
            for k in range(4):
                nc.tensor.matmul(g[:], ft[:, k, :], WihT[:, k, sl],
                                 start=False, stop=False)
            yield 1
            for k in range(4, 8):
                nc.tensor.matmul(g[:], ft[:, k, :], WihT[:, k, sl],
                                 start=False, stop=False)
            for k in range(4):
                nc.tensor.matmul(g[:], hT[:, k, :], WhhT[:, k, sl],
                                 start=False, stop=(k == 3))
            if blk == 0:      # f
                sf = step.tile([128, HID], BF16, tag='lsf', bufs=1)
                nc.scalar.activation(sf[:], g[:], AF.Sigmoid)
                m1 = step.tile([128, HID], BF16, tag='lm1', bufs=1)
                nc.vector.tensor_tensor(m1[:], sf[:], cst[:], op=ALU.mult)
                tiles['m1'] = m1
            elif blk == 1:    # i
                si_ = step.tile([128, HID], BF16, tag='lsi', bufs=1)
                nc.scalar.activation(si_[:], g[:], AF.Sigmoid)
                tiles['si'] = si_
            elif blk == 2:    # g
                tg = step.tile([128, HID], BF16, tag='ltg', bufs=1)
                nc.scalar.activation(tg[:], g[:], AF.Tanh)
                m2 = step.tile([128, HID], BF16, tag='lm2', bufs=1)
                nc.gpsimd.tensor_tensor(m2[:], tiles['si'][:], tg[:],
                                        op=ALU.mult)
                nc.vector.tensor_tensor(cst[:], tiles['m1'][:], m2[:],
                                        op=ALU.add)
                tct = step.tile([128, HID], BF16, tag='ltct', bufs=1)
                nc.scalar.activation(tct[:], cst[:], AF.Tanh)
                tiles['tct'] = tct
            else:             # o
                so = step.tile([128, HID], BF16, tag='lsf', bufs=1)
                nc.scalar.activation(so[:], g[:], AF.Sigmoid)
                hsb = step.tile([128, HID], BF16, tag='lhsb', bufs=1)
                nc.vector.tensor_tensor(hsb[:], so[:], tiles['tct'][:],
                                        op=ALU.mult)
            yield 1
        # prefetch next step's features while pointwise runs
        if t + 1 < B:
            ft = dma2.tile([128, 8, 128], BF16, tag='lft', bufs=1)
            nc.gpsimd.dma_start(
                ft[:], din['featT_full'][:, t + 1, :]
                .rearrange("(c p) n -> p c n", p=128))
        for c in range(4):
            o = 80 + 128 * (c % 2)
            nc.tensor.transpose(axb[:, o:o + 128],
                                hsb[:, c * 128:(c + 1) * 128], ident[:])
            nc.vector.tensor_copy(hT[:, c, :], axb[:, o:o + 128])
        nc.sync.dma_start(lstmT_dram[t, :, :, :], hT[:])
        yield 1


def _rec_h1(ctx, l, i):
    """First half of a recurrence step: prefetches + attention chain
    (everything up to the diag softmax-weight scatter).  Returns the tiles
    the second half needs."""
    nc = ctx['nc']
    step, dma2 = ctx['step'], ctx['dma2']
    ident = ctx['ident']
    Kneg = ctx['Kneg'][l]
    ew0, ew1 = ctx['ew0'][l], ctx['ew1'][l]
    Wz0d, Wz1d = ctx['Wz0d'][l], ctx['Wz1d'][l]
    pre_dram_l = ctx['pre_dram'][l]
    din = ctx['din']
    axb = ctx['axb']
    so = 336 + 48 * l        # this layer's transpose scratch cols in axb

    # ---------- prefetches (spread across SP / Pool / ACT queues) ----------
    pa = dma2.tile([128, HID], BF16, tag='pa', bufs=3)
    for s4 in range(4):
        eng = nc.sync if s4 < 2 else nc.gpsimd
        eng.dma_start(pa[32 * s4:32 * s4 + 4, :],
                      pre_dram_l[i, 4 * s4:4 * s4 + 4, :])
    ia = step.tile([36, HID], BF16, tag='ia', bufs=3)
    nc.sync.dma_start(ia[0:4, :], pre_dram_l[i, 16:20, :])
    pb = step.tile([128, HID], BF16, tag='pb', bufs=3)
    nc.gpsimd.dma_start(pb[96:100, :], pre_dram_l[i, 20:24, :])
    mq = step.tile([36, HID], BF16, tag='mq', bufs=3)
    nc.gpsimd.dma_start(mq[32:36, :], ctx['hq_dram'][l][i, :, :])
    out = dict(mq=mq, ia=ia, pb=pb, pa=pa)
    if i > 0:
        absr = dma2.tile([BL, 2 * N], F32, tag='absr', bufs=3)
        nc.sync.dma_start(absr[:], din['absr'][:, i, :])
        qpt = dma2.tile([BL, 1], F32, tag='qpt', bufs=3)
        nc.sync.dma_start(qpt[:], ctx['qpre_dram'][l][i, :])
        yield out

        # ---------- attention ----------
        aneg = step.tile([BL, N], F32, tag='aw', bufs=4)
        nc.vector.scalar_tensor_tensor(
            aneg[:, 0:i], Kneg[:, 0:i], qpt[:, 0:1], absr[:, 0:i],
            op0=ALU.subtract, op1=ALU.subtract)
        mneg = step.tile([BL, 1], F32, tag='mn', bufs=2)
        nc.vector.tensor_reduce(mneg[:], aneg[:, 0:i], axis=AX.X, op=ALU.min)
        negm = step.tile([BL, 1], F32, tag='nm', bufs=2)
        nc.vector.tensor_scalar_mul(negm[:], mneg[:], -1.0)
        yield out
        sgt = step.tile([BL, N], F32, tag='aw', bufs=4)
        nc.scalar.activation(sgt[:, 0:i], aneg[:, 0:i], AF.Sigmoid,
                             bias=negm[:], scale=1.0)
        yield out
        rt = step.tile([BL, N], F32, tag='aw', bufs=4)
        nc.vector.reciprocal(rt[:, 0:i], sgt[:, 0:i])
        e_t = step.tile([BL, N], F32, tag='aw', bufs=4)
        zs = step.tile([BL, 1], F32, tag='zs', bufs=2)
        nc.vector.tensor_scalar(e_t[:, 0:i], rt[:, 0:i], 1.0, 0.0,
                                op0=ALU.subtract, op1=ALU.add, accum_out=zs[:])
        yield out
        rz = step.tile([BL, 1], F32, tag='rz', bufs=2)
        nc.vector.reciprocal(rz[:], zs[:])
        ewn = step.tile([BL, N], F32, tag='aw', bufs=4)
        nc.vector.tensor_scalar_mul(ewn[:, 0:i], e_t[:, 0:i], rz[:])
        yield out
        # split normalized weights by s-mask
        nc.vector.tensor_tensor(ew0[:, 0:i], ewn[:, 0:i], absr[:, N:N + i],
                                op=ALU.mult)
        nc.vector.tensor_tensor(ew1[:, 0:i], ewn[:, 0:i], ew0[:, 0:i],
                                op=ALU.subtract)
        yield out
        nc.tensor.transpose(axb[:, so:so + 4], ew0[:], ident[0:BL, 0:BL])
        nc.tensor.transpose(axb[:, so + 4:so + 8], ew1[:],
                            ident[0:BL, 0:BL])
        nc.vector.tensor_copy(_diag(Wz0d), axb[:, so:so + 4])
        nc.vector.tensor_copy(_diag(Wz1d), axb[:, so + 4:so + 8])
    yield out


def _rec_h2(ctx, l, i, h1):
    """Second half: M, gates, combine, V/K projections."""
    nc = ctx['nc']
    step = ctx['step']
    ident, ones_row = ctx['ident'], ctx['ones_row']
    HTl, HTn = ctx['HT'][l], ctx['HT'][l + 1]
    V01, Kneg = ctx['V01'][l], ctx['Kneg'][l]
    Wz0d, Wz1d = ctx['Wz0d'][l], ctx['Wz1d'][l]
    Wc1, Wc2, Wrt = ctx['Wc1'][l], ctx['Wc2'][l], ctx['Wrt'][l]
    b2 = ctx['b2'][l]
    GA = ctx['gA'].tile([128, HID], F32, tag=f'GA{l}')
    GB = ctx['gB'].tile([128, HID], F32, tag=f'GB{l}')
    GV = ctx['gV'].tile([128, HID], F32, tag='GV')
    axb = ctx['axb']
    so = 336 + 48 * l
    mq, ia, pb, pa = h1['mq'], h1['ia'], h1['pb'], h1['pa']
    if i > 0:
        # ------- M: sum_b diag-weights @ V (accumulates into GB[96:100]) ----
        for b in range(BL):
            nc.tensor.matmul(GB[96:100, :], Wz0d[:, 4 * b:4 * b + 4],
                             V01[:, b, 0:HID], start=(b == 0), stop=False,
                             tile_position=(0, 96))
        for b in range(BL):
            nc.tensor.matmul(GB[96:100, :], Wz1d[:, 4 * b:4 * b + 4],
                             V01[:, b, HID:H2], start=False, stop=(b == 3),
                             tile_position=(0, 96))
        yield
        # mq rows 0:4 = M (already normalized via ewn)
        nc.vector.tensor_tensor(mq[0:4, :], GB[96:100, :],
                                ctx['zrow'][96:100, :], op=ALU.add)
        yield
        # MT = M transposed [128, 16]
        for c in range(4):
            nc.tensor.transpose(axb[:, so + 8 + 4 * c:so + 12 + 4 * c],
                                mq[0:4, 128 * c:128 * (c + 1)],
                                ident[0:BL, 0:BL])
        MT = step.tile([128, 4, 16], FP8, tag='MT', bufs=3)
        nc.vector.tensor_copy(MT[:, :, 0:4], axb[:, so + 8:so + 24])
    else:
        MT = step.tile([128, 4, 16], FP8, tag='MT', bufs=3)
        nc.vector.memset(MT[:], 0.0)
        nc.vector.memset(mq[0:4, :], 0.0)
    yield

    # ---------- gates ----------
    # GA bands: zC@0:4  zP@32:36  rC@64:68  rP@96:100   (h-side sigma + q-side)
    for s4 in range(4):
        r0 = 32 * s4
        for k in range(4):
            nc.tensor.matmul(GA[r0:r0 + 4, :], MT[:, k, 0:4],
                             Wc1[:, k, HID * s4:HID * (s4 + 1)],
                             start=(k == 0), stop=(k == 3),
                             tile_position=(0, r0))
    # GB bands: nC@64:68, nP@32:36 (h-side n-gate + bias)
    for si, r0 in enumerate((64, 32)):
        nc.tensor.matmul(GB[r0:r0 + 4, :], ones_row[0:1, 0:4],
                         b2[:, HID * si:HID * (si + 1)],
                         start=True, stop=False, tile_position=(0, r0))
        for k in range(4):
            nc.tensor.matmul(GB[r0:r0 + 4, :], MT[:, k, 0:4],
                             Wc2[:, k, HID * si:HID * (si + 1)],
                             start=False, stop=(k == 3), tile_position=(0, r0))
    yield
    Asg = step.tile([128, HID], BF16, tag='Ag', bufs=2)
    nc.vector.tensor_tensor(Asg[0:100, :], GA[0:100, :], pa[0:100, :],
                            op=ALU.add)
    Asig = step.tile([128, HID], BF16, tag='As', bufs=2)
    nc.scalar.activation(Asig[0:100, :], Asg[0:100, :], AF.Sigmoid)
    yield
    # ia rows 32:36 <- psB nP (M-side i_n);  pb rows 32:36 = q-side h_n (DMA)
    nc.scalar.activation(ia[32:36, :], GB[32:36, :], AF.Copy)
    ntin = step.tile([36, HID], BF16, tag='ns', bufs=3)
    nc.vector.tensor_tensor(ntin[0:4, :], Asig[64:68, :], GB[64:68, :],
                            op=ALU.mult)
    nc.vector.tensor_tensor(ntin[32:36, :], Asig[96:100, :], pb[96:100, :],
                            op=ALU.mult)
    nc.vector.tensor_tensor(ntin[:], ntin[:], ia[:], op=ALU.add)
    yield
    Nt = step.tile([36, HID], BF16, tag='ns', bufs=3)
    nc.scalar.activation(Nt[:], ntin[:], AF.Tanh)
    yield
    # ---------- combine: h = nC + zC(M-nC) + nP + zP(q-nP) ----------
    sdf = step.tile([36, HID], BF16, tag='ns', bufs=3)
    nc.vector.tensor_tensor(sdf[:], mq[:], Nt[:], op=ALU.subtract)
    zm = step.tile([36, HID], BF16, tag='zh', bufs=3)
    nc.vector.tensor_tensor(zm[:], Asig[0:36, :], sdf[:], op=ALU.mult)
    yield
    hpC = step.tile([BL, HID], BF16, tag='zh', bufs=3)
    nc.vector.tensor_tensor(hpC[:], Nt[0:4, :], zm[0:4, :], op=ALU.add)
    hpP = step.tile([BL, HID], BF16, tag='zh', bufs=3)
    nc.vector.tensor_tensor(hpP[:], Nt[32:36, :], zm[32:36, :], op=ALU.add)
    hbf = step.tile([BL, HID], BF16, tag='hb', bufs=2)
    nc.vector.tensor_tensor(hbf[:], hpC[:], hpP[:], op=ALU.add)
    yield

    # ---------- h transposes + V/K ----------
    for c in range(4):
        nc.tensor.transpose(axb[:, so + 24 + 4 * c:so + 28 + 4 * c],
                            hbf[:, 128 * c:128 * (c + 1)], ident[0:BL, 0:BL])
    nc.vector.tensor_copy(HTn[:, :, :, i], axb[:, so + 24:so + 40])
    yield
    hTn = [HTn[:, k, :, i] for k in range(4)]
    for k in range(4):
        nc.tensor.matmul(GV[0:4, :], hTn[k], Wrt[:, k, 0:HID],
                         start=(k == 0), stop=(k == 3), tile_position=(0, 0))
    for k in range(4):
        nc.tensor.matmul(GV[32:36, :], hTn[k], Wrt[:, k, HID:H2],
                         start=(k == 0), stop=(k == 3), tile_position=(0, 32))
    for k in range(4):
        nc.tensor.matmul(GV[64:68, 0:1], hTn[k], Wrt[:, k, H2:H2 + 1],
                         start=(k == 0), stop=(k == 3), tile_position=(0, 64))
    nc.vector.tensor_tensor(Kneg[:, i:i + 1], GV[64:68, 0:1],
                            ctx['ones4c'][64:68, 0:1], op=ALU.mult)
    yield
    Vsb = step.tile([BL, H2], BF16, tag='Vs', bufs=1)
    nc.scalar.activation(Vsb[:, 0:HID], GV[0:4, :], AF.Copy)
    nc.vector.tensor_tensor(Vsb[:, HID:H2], GV[32:36, :],
                            ctx['zrow'][32:36, :], op=ALU.add)
    nc.gpsimd.dma_start(V01[i:i + 1, :, :], Vsb[:])
    if l + 1 < L:
        nc.sync.dma_start(ctx['hq_dram'][l + 1][i, :, :], hbf[:])
    if ctx['dbg_dram'] is not None:
        hdb = step.tile([BL, HID], F32, tag='hd', bufs=2)
        nc.vector.tensor_copy(hdb[:], hbf[:])
        nc.sync.dma_start(ctx['dbg_dram'][l, i, :, :], hdb[:])


def _final_mlp(nc, tc, din, wpool, step, ps, featT4, HT, lstmTl, ones_row,
               ident, out_dram):
    mlp0T = _loadw(nc, wpool, din['mlp0T'], 4 * HID + EMB, HID, 'mlp0T')
    mlp1T = _loadw(nc, wpool, din['mlp1T'], HID, HID, 'mlp1T')
    outWT = _loadw(nc, wpool, din['outWT'], HID, 8, 'outWT')
    b0 = wpool.tile([1, HID], BF16, tag='mb0')
    nc.gpsimd.dma_start(b0[:], din['mlp0_b'][:])
    b1 = wpool.tile([1, HID], BF16, tag='mb1')
    nc.gpsimd.dma_start(b1[:], din['mlp1_b'][:])
    bo = wpool.tile([1, 8], BF16, tag='mbo')
    nc.gpsimd.dma_start(bo[:], din['out_b'][:])

    for b in range(BL):
        p1 = ps.tile([128, HID], F32, tag='fp1')
        kc = 0
        for blk in range(3):
            for c in range(4):
                nc.tensor.matmul(p1[:], HT[blk][:, c, b, :], mlp0T[:, kc, :],
                                 start=(kc == 0), stop=False)
                kc += 1
        for k in range(8):
            nc.tensor.matmul(p1[:], featT4[:, k, b, :], mlp0T[:, kc, :],
                             start=False, stop=False)
            kc += 1
        for c in range(4):
            nc.tensor.matmul(p1[:], lstmTl[:, c, b, :], mlp0T[:, kc, :],
                             start=False, stop=False)
            kc += 1
        nc.tensor.matmul(p1[:], ones_row[0:1, 0:128], b0[:],
                         start=False, stop=True)
        x1 = step.tile([128, HID], BF16, tag='mx1', bufs=2)
        nc.scalar.activation(x1[:], p1[:], AF.Relu)
        x1T = step.tile([128, 4, 128], BF16, tag='mx1T', bufs=2)
        for c in range(4):
            tp = ps.tile([128, 128], BF16, tag='ftp')
            nc.tensor.transpose(tp[:], x1[:, 128 * c:128 * (c + 1)], ident[:])
            nc.vector.tensor_copy(x1T[:, c, :], tp[:])
        p2 = ps.tile([128, HID], F32, tag='fp2')
        for k in range(4):
            nc.tensor.matmul(p2[:], x1T[:, k, :], mlp1T[:, k, :],
                             start=(k == 0), stop=False)
        nc.tensor.matmul(p2[:], ones_row[0:1, 0:128], b1[:],
                         start=False, stop=True)
        x2 = step.tile([128, HID], BF16, tag='mx2', bufs=2)
        nc.scalar.activation(x2[:], p2[:], AF.Relu)
        x2T = step.tile([128, 4, 128], BF16, tag='mx2T', bufs=2)
        for c in range(4):
            tp = ps.tile([128, 128], BF16, tag='ftp')
            nc.tensor.transpose(tp[:], x2[:, 128 * c:128 * (c + 1)], ident[:])
            nc.vector.tensor_copy(x2T[:, c, :], tp[:])
        po = ps.tile([128, 8], F32, tag='fpo')
        for k in range(4):
            nc.tensor.matmul(po[:], x2T[:, k, :], outWT[:, k, :],
                             start=(k == 0), stop=False)
        nc.tensor.matmul(po[:], ones_row[0:1, 0:128], bo[:],
                         start=False, stop=True)
        ot = step.tile([128, NCLS], F32, tag='mot', bufs=2)
        nc.vector.tensor_copy(ot[:], po[:, 0:NCLS])
        nc.sync.dma_start(out_dram[b, :, :], ot[:])


# ================================================================ entry point

_NC_CACHE = {}


def kernel(**inputs):
    maps = prep_inputs(inputs)
    if 'nc' not in _NC_CACHE:
        _NC_CACHE['nc'] = build_nc()
    nc = _NC_CACHE['nc']
    res = run_bass_kernel_spmd(nc, maps, list(range(NCORES)))
    out = np.concatenate([res.results[c]['out'] for c in range(NCORES)], axis=0)
    return out.astype(np.float32)
